# revision 21
# baseline (speedup 1.0000x reference)
"""Fused quantized BasicBlock (1-bit weights / 4-bit acts) for TRN2, 8-core data-parallel.

Math: both convs see integer activations k in {0..15} (exactly representable in
fp8e4) and sign weights in {-1,0,+1}; the 3x3 conv is 9 shifted DoubleRow fp8
matmuls (K=256 contraction in one pass) accumulating exactly in fp32 PSUM.
All scalings (LSQ alpha, IR-Net weight scale, BN affine) fold into a
per-output-channel affine applied in the epilogue.

Layout: activations live in SBUF as fp8 integers in a zero-padded [60 x 58]
image (data at rows/cols 1..56, stride 58). A 3x3 tap (kh,kw) is then just a
contiguous 464-byte-per-k-chunk slice at offset (r0+kh)*58+kw covering 8 output
rows; the 2 pad columns per row produce garbage PSUM columns that the epilogue
never reads.
"""

import numpy as np
import ml_dtypes

import concourse.bass as bass
import concourse.bacc as bacc
import concourse.mybir as mybir
from concourse.tile import TileContext
from concourse.bass_utils import run_bass_kernel_spmd

F32 = mybir.dt.float32
FP8 = mybir.dt.float8e4
NP_FP8 = ml_dtypes.float8_e4m3
AF = mybir.ActivationFunctionType
ALU = mybir.AluOpType
DR = mybir.MatmulPerfMode.DoubleRow

B, C, H, W = 32, 256, 56, 56
N_CORES = 8
BPC = B // N_CORES          # images per core
PW, PH = 58, 60             # padded image: 60 rows x 58 cols, data at [1:57, 1:57]
KCH = 3520                  # bytes per k-chunk (>= PH*PW, multiple of 16 for DoubleRow)
KCH_A = 1568                # image-0 top band: padded rows 0..25 + wrap row 26
KCH_B = 2032                # image-0 bottom band: padded rows 24..57 + wrap row (local 0..34)
NMM = 8 * PW                # moving free dim per matmul: 8 output rows
MAGIC = float(np.float32(2.0 ** 23))  # fp32 add/sub of 2^23 == round-to-nearest-even
QMAX = 15.0

_module_cache = {}


def _emit_memset_pads(nc, kt):
    """Zero the padding borders of one [128, 2, KCH] activation tile."""
    for cc in (0, 1):
        v = kt[:, cc, :]
        nc.vector.memset(v[:, 0:PW], 0.0)                    # row 0
        nc.vector.memset(v[:, 57 * PW:PH * PW], 0.0)         # rows 57..59
        vv = v[:, 0:PH * PW].rearrange("p (r c) -> p r c", c=PW)
        nc.vector.memset(vv[:, 1:57, 0:1], 0.0)              # col 0
        nc.vector.memset(vv[:, 1:57, 57:58], 0.0)            # col 57


def _emit_x_dma(nc, xr, i, xpool, nrows, tag, bufs):
    """Issue the DMAs for image i's input, in row-chunks of nrows.
    bufs must cover all chunks in flight so triggers don't stall on slots."""
    tiles = {}
    for cc in (0, 1):
        for rr0 in range(0, H, nrows):
            xt = xpool.tile([128, nrows * W], F32, tag=tag, bufs=bufs,
                            name=f"x_{i}_{cc}_{rr0}")
            nc.sync.dma_start(out=xt[:], in_=xr[i, cc][:, rr0 * W:(rr0 + nrows) * W])
            tiles[(cc, rr0)] = xt
    return tiles


def _emit_quant_input(nc, i, xtiles, tqp, rqp, k1t, coef_t, nrows):
    """k1 = min(rne(relu(x/alpha1)), 15) as fp8 into padded layout."""
    for cc in (0, 1):
        for rr0 in range(0, H, nrows):
            xt = xtiles[(cc, rr0)]
            tq = tqp.tile([128, nrows * W], F32, tag="tq")
            nc.scalar.activation(out=tq[:], in_=xt[:],
                                 func=AF.Relu, scale=coef_t[:, 8:9])
            rq = rqp.tile([128, nrows * W], F32, tag="rq")
            nc.vector.tensor_scalar(
                out=rq[:], in0=tq[:], scalar1=MAGIC, scalar2=MAGIC,
                op0=ALU.add, op1=ALU.subtract)
            dst = k1t[i][:, cc, 0:PH * PW].rearrange("p (r c) -> p r c", c=PW)[
                :, rr0 + 1:rr0 + 1 + nrows, 1:57]
            nc.vector.tensor_scalar_min(
                dst, rq[:].rearrange("p (r c) -> p r c", c=W), QMAX)


def _emit_conv(nc, i, wt, kin, psum, layer2, ep1p, ep2p, stp, k2t, o_r, coef_t,
               kin_banded=None):
    """One 3x3 conv layer for image i: 9 shifted DoubleRow matmuls per output tile.

    kin_banded: optional rb -> (tile, base_padded_row) override so early row
    blocks can depend on a partially-quantized input (startup pipelining)."""
    if layer2 and i == BPC - 1:
        # split the very last row block so the final post-matmul epilogue+DMA
        # (serial tail after the last MM) is as small as possible
        blocks = [(r0, 8) for r0 in range(0, 48, 8)] + [(48, 6), (54, 2)]
    else:
        blocks = [(r0, 8) for r0 in range(0, 56, 8)]
    for r0, nr in blocks:
        for occ in (0, 1):
            if kin_banded is not None:
                ktile, base = kin_banded(r0 // 8)
            else:
                ktile, base = kin[i], 0
            nmm = nr * PW
            ps = psum.tile([128, NMM], F32, tag="ps")
            for off in range(9):
                kh, kw = divmod(off, 3)
                s = (r0 + kh - base) * PW + kw
                nc.tensor.matmul(
                    ps[:, 0:nmm], wt[occ][:, :, off, :], ktile[:, :, s:s + nmm],
                    start=(off == 0), stop=(off == 8), perf_mode=DR)
            psv = ps[:, 0:nmm].rearrange("p (r c) -> p r c", c=PW)[:, :, 0:56]
            if not layer2:
                # k2 = min(rne(relu((A1/a2)*conv + B1/a2)), 15) -> fp8, all on DVE
                # (ACT is the scarcer engine: it owns input quant + final epilogue)
                t1 = ep1p.tile([128, 8 * 56], F32, tag="ep1")
                nc.vector.tensor_scalar(
                    out=t1[:, 0:nr * 56].rearrange("p (r c) -> p r c", c=56), in0=psv,
                    scalar1=coef_t[:, occ:occ + 1], scalar2=coef_t[:, 2 + occ:3 + occ],
                    op0=ALU.mult, op1=ALU.add)
                t2 = ep2p.tile([128, 8 * 56], F32, tag="ep2")
                nc.vector.tensor_scalar(
                    out=t2[:, 0:nr * 56], in0=t1[:, 0:nr * 56], scalar1=0.0,
                    scalar2=MAGIC, op0=ALU.max, op1=ALU.add)
                dst = k2t[i][:, occ, 0:PH * PW].rearrange("p (r c) -> p r c", c=PW)[
                    :, r0 + 1:r0 + 1 + nr, 1:57]
                nc.vector.tensor_scalar(
                    out=dst,
                    in0=t2[:, 0:nr * 56].rearrange("p (r c) -> p r c", c=56),
                    scalar1=MAGIC, scalar2=QMAX,
                    op0=ALU.subtract, op1=ALU.min)
            else:
                # out = relu(A2*conv + B2) on ACT, then DMA to DRAM
                st = stp.tile([128, 8 * 56], F32, tag="st")
                nc.scalar.activation(
                    out=st[:, 0:nr * 56].rearrange("p (r c) -> p r c", c=56), in_=psv,
                    func=AF.Relu, scale=coef_t[:, 4 + occ:5 + occ],
                    bias=coef_t[:, 6 + occ:7 + occ])
                nc.sync.dma_start(
                    out=o_r[i, occ][:, r0 * 56:(r0 + nr) * 56], in_=st[:, 0:nr * 56])


def _build_module():
    # Bacc (not raw Bass): its compile() legalizes multi-sem waits (TRN2 allows
    # one wait per instruction) and inserts activation table loads.
    nc = bacc.Bacc("TRN2", debug=False)
    x_d = nc.dram_tensor("x", [BPC, C, H, W], F32, kind="ExternalInput")
    w1_d = nc.dram_tensor("w1p", [2, 128, 2, 9, 128], FP8, kind="ExternalInput")
    w2_d = nc.dram_tensor("w2p", [2, 128, 2, 9, 128], FP8, kind="ExternalInput")
    cf_d = nc.dram_tensor("coef", [128, 9], F32, kind="ExternalInput")
    o_d = nc.dram_tensor("out", [BPC, C, H, W], F32, kind="ExternalOutput")

    xr = x_d.ap().rearrange("b (cc p) h w -> b cc p (h w)", p=128)
    o_r = o_d.ap().rearrange("b (cc p) h w -> b cc p (h w)", p=128)

    with TileContext(nc) as tc:
        with tc.tile_pool(name="weights", bufs=1) as wpool, \
             tc.tile_pool(name="acts", bufs=1) as kpool, \
             tc.tile_pool(name="xin", bufs=4) as xpool, \
             tc.tile_pool(name="tq", bufs=2) as tqp, \
             tc.tile_pool(name="rq", bufs=2) as rqp, \
             tc.tile_pool(name="ep1", bufs=4) as ep1p, \
             tc.tile_pool(name="ep2", bufs=4) as ep2p, \
             tc.tile_pool(name="st", bufs=4) as stp, \
             tc.tile_pool(name="coef", bufs=1) as cfp, \
             tc.tile_pool(name="psum", bufs=8, space="PSUM") as psum:

            # coef first (tiny), then image 0's top-band input chunks: nothing
            # else ahead of them in the DMA queues — the first matmul gates
            # on image 0's top band.
            coef_t = cfp.tile([128, 9], F32, tag="coef")
            nc.sync.dma_start(out=coef_t[:], in_=cf_d.ap())

            # image 0's critical top-band chunks spread over THREE independent
            # DMA paths (each trigger engine's transfers serialize on its own
            # queue): 2 on the ACT HWDGE queue, 1 on Sync HWDGE, 1 on SWDGE.
            x0 = {}
            for rr0, eng in (((0, nc.scalar)), ((14, nc.scalar))):
                xt = xpool.tile([128, 14 * W], F32, tag="xin0", bufs=8,
                                name=f"x_0_0_{rr0}")
                eng.dma_start(out=xt[:], in_=xr[0, 0][:, rr0 * W:(rr0 + 14) * W])
                x0[(0, rr0)] = xt
            for rr0, eng in (((0, nc.sync)), ((14, nc.gpsimd))):
                xt = xpool.tile([128, 14 * W], F32, tag="xin0", bufs=8,
                                name=f"x_0_1_{rr0}")
                eng.dma_start(out=xt[:], in_=xr[0, 1][:, rr0 * W:(rr0 + 14) * W])
                x0[(1, rr0)] = xt

            # dummy activation gated only on the coef DMA: pulls the one-time
            # ACT_TABLE_LOAD (~1.3us) off the quant critical path.
            scr = cfp.tile([128, 1], F32, tag="scr")
            nc.scalar.activation(out=scr[:], in_=coef_t[:, 0:1], func=AF.Relu)

            w1t, w2t = [], []
            for occ in (0, 1):
                t = wpool.tile([128, 2, 9, 128], FP8, tag=f"w1_{occ}", name=f"w1_{occ}")
                nc.sync.dma_start(out=t[:], in_=w1_d.ap()[occ])
                w1t.append(t)

            for rr0 in (28, 42):
                for cc in (0, 1):
                    xt = xpool.tile([128, 14 * W], F32, tag="xin0", bufs=8,
                                    name=f"x_0_{cc}_{rr0}")
                    nc.sync.dma_start(
                        out=xt[:], in_=xr[0, cc][:, rr0 * W:(rr0 + 14) * W])
                    x0[(cc, rr0)] = xt

            # image 0's layer-1 input lives in two band tiles so top-row
            # matmuls start before the bottom half is quantized.
            k1a = kpool.tile([128, 2, KCH_A], FP8, tag="k1a", name="k1a")
            k1b = kpool.tile([128, 2, KCH_B], FP8, tag="k1b", name="k1b")
            k1t, k2t = [None], []
            for i in range(BPC):
                if i > 0:
                    k1t.append(kpool.tile([128, 2, KCH], FP8, tag=f"k1_{i}",
                                          name=f"k1_{i}"))
                k2t.append(kpool.tile([128, 2, KCH], FP8, tag=f"k2_{i}", name=f"k2_{i}"))

            def av(cc, lo, hi):
                return k1a[:, cc, 0:27 * PW].rearrange("p (r c) -> p r c", c=PW)[
                    :, lo:hi, 1:57]

            def bv(cc, lo, hi):
                return k1b[:, cc, 0:35 * PW].rearrange("p (r c) -> p r c", c=PW)[
                    :, lo:hi, 1:57]

            # band pad memsets
            for cc in (0, 1):
                va = k1a[:, cc, :]
                nc.vector.memset(va[:, 0:PW], 0.0)                 # padded row 0
                nc.vector.memset(va[:, 26 * PW:KCH_A], 0.0)        # wrap row + slack
                vva = va[:, 0:27 * PW].rearrange("p (r c) -> p r c", c=PW)
                nc.vector.memset(vva[:, 1:26, 0:1], 0.0)
                nc.vector.memset(vva[:, 1:26, 57:58], 0.0)
                vb = k1b[:, cc, :]
                nc.vector.memset(vb[:, 33 * PW:KCH_B], 0.0)        # rows 57+, slack
                vvb = vb[:, 0:35 * PW].rearrange("p (r c) -> p r c", c=PW)
                nc.vector.memset(vvb[:, 0:33, 0:1], 0.0)
                nc.vector.memset(vvb[:, 0:33, 57:58], 0.0)

            # quantize image 0 chunk-by-chunk into the bands (cc interleaved so
            # the top band completes as early as possible)
            chunk_dsts = {
                0:  lambda cc: [(av(cc, 1, 15), 0, 14)],
                14: lambda cc: [(av(cc, 15, 26), 0, 11), (bv(cc, 0, 5), 9, 14)],
                28: lambda cc: [(bv(cc, 5, 19), 0, 14)],
                42: lambda cc: [(bv(cc, 19, 33), 0, 14)],
            }
            for rr0 in (0, 14, 28, 42):
                for cc in (0, 1):
                    tq = tqp.tile([128, 14 * W], F32, tag="tq")
                    nc.scalar.activation(out=tq[:], in_=x0[(cc, rr0)][:],
                                         func=AF.Relu, scale=coef_t[:, 8:9])
                    rq = rqp.tile([128, 14 * W], F32, tag="rq")
                    nc.vector.tensor_scalar(
                        out=rq[:], in0=tq[:], scalar1=MAGIC, scalar2=MAGIC,
                        op0=ALU.add, op1=ALU.subtract)
                    rqv = rq[:].rearrange("p (r c) -> p r c", c=W)
                    for view, lo, hi in chunk_dsts[rr0](cc):
                        nc.vector.tensor_scalar_min(view, rqv[:, lo:hi], QMAX)
            _emit_memset_pads(nc, k2t[0])

            def quant(i, xtiles, nrows):
                _emit_memset_pads(nc, k1t[i])
                _emit_quant_input(nc, i, xtiles, tqp, rqp, k1t, coef_t, nrows)
                _emit_memset_pads(nc, k2t[i])

            def l1(i):
                banded = (lambda rb: (k1a, 0) if rb < 3 else (k1b, 24)) \
                    if i == 0 else None
                _emit_conv(nc, i, w1t, k1t, psum, False, ep1p, ep2p, stp,
                           k2t, o_r, coef_t, kin_banded=banded)

            def l2(i):
                _emit_conv(nc, i, w2t, k2t, psum, True, ep1p, ep2p, stp,
                           None, o_r, coef_t)

            # stagger so PE never waits: image i's L1 can start while image
            # i+1 still quantizes; L2(i) runs after L1(i)'s epilogues.
            x1 = _emit_x_dma(nc, xr, 1, xpool, 28, 'xin', 4)
            for occ in (0, 1):
                t = wpool.tile([128, 2, 9, 128], FP8, tag=f"w2_{occ}", name=f"w2_{occ}")
                nc.sync.dma_start(out=t[:], in_=w2_d.ap()[occ])
                w2t.append(t)
            quant(1, x1, 28)
            l1(0)
            x2 = _emit_x_dma(nc, xr, 2, xpool, 28, 'xin', 4)
            quant(2, x2, 28)
            l1(1); l2(0)
            x3 = _emit_x_dma(nc, xr, 3, xpool, 28, 'xin', 4)
            quant(3, x3, 28)
            l1(2); l2(1)
            l1(3); l2(2)
            l2(3)

    nc.compile()
    return nc


def get_module():
    if "nc" not in _module_cache:
        _module_cache["nc"] = _build_module()
    return _module_cache["nc"]


def _binarize(w):
    """IR-Net forward: sign(normalized w) and per-out-channel scale (fp32)."""
    w = np.asarray(w, np.float32)
    mu = w.mean(axis=(1, 2, 3), keepdims=True, dtype=np.float32)
    var = ((w - mu) ** 2).mean(axis=(1, 2, 3), keepdims=True, dtype=np.float32)
    std = np.sqrt(var)
    wn = (w - mu) / (std + np.float32(1e-5))
    sgn = np.sign(wn).astype(np.float32)
    scale = np.abs(wn).mean(axis=(1, 2, 3), dtype=np.float32)  # [O]
    return sgn, scale


def _pack_weights(sgn):
    """[O=256, C=256, 3, 3] signs -> [occ, p(Ki), h(Ko), off, m] fp8 with c = h*128+p."""
    s = sgn.reshape(256, 256, 9)
    s = s.reshape(2, 128, 2, 128, 9)            # [occ, m, h, p, off]
    s = np.transpose(s, (0, 3, 2, 4, 1))        # [occ, p, h, off, m]
    return np.ascontiguousarray(s).astype(NP_FP8)


def kernel(x, w1, alpha1, g1, b1, m1, v1, w2, alpha2, g2, b2, m2, v2,
           _trace=False):
    f32 = np.float32
    x = np.asarray(x, f32)
    a1 = f32(np.asarray(alpha1).reshape(()))
    a2 = f32(np.asarray(alpha2).reshape(()))
    g1, b1, m1, v1 = (np.asarray(t, f32) for t in (g1, b1, m1, v1))
    g2, b2, m2, v2 = (np.asarray(t, f32) for t in (g2, b2, m2, v2))

    s1, sc1 = _binarize(w1)
    s2, sc2 = _binarize(w2)
    inv1 = g1 / np.sqrt(v1 + f32(1e-5))
    inv2 = g2 / np.sqrt(v2 + f32(1e-5))

    A1 = (a1 * sc1 * inv1 / a2).astype(f32)         # folds layer2 1/alpha in
    B1 = ((b1 - m1 * inv1) / a2).astype(f32)
    A2 = (a2 * sc2 * inv2).astype(f32)
    B2 = (b2 - m2 * inv2).astype(f32)

    coef = np.zeros((9, 128), f32)
    coef[0:2] = A1.reshape(2, 128)
    coef[2:4] = B1.reshape(2, 128)
    coef[4:6] = A2.reshape(2, 128)
    coef[6:8] = B2.reshape(2, 128)
    coef[8] = f32(1.0) / a1

    coef = np.ascontiguousarray(coef.T)   # [128, 9]: contiguous per-partition DMA

    w1p = _pack_weights(s1)
    w2p = _pack_weights(s2)

    nc = get_module()
    in_maps = [
        {"x": np.ascontiguousarray(x[i * BPC:(i + 1) * BPC]),
         "w1p": w1p, "w2p": w2p, "coef": coef}
        for i in range(N_CORES)
    ]
    res = run_bass_kernel_spmd(nc, in_maps, core_ids=list(range(N_CORES)),
                               trace=_trace)
    out = np.concatenate([r["out"] for r in res.results], axis=0)
    if _trace:
        return out, res
    return out
